# revision 1
# baseline (speedup 1.0000x reference)
"""NCC loss (9x9x9 box normalized cross-correlation) on 8 TRN2 NeuronCores.

Inputs: y_pred, y_true f32 (2,1,128,128,128). Output: scalar f32 loss.

Sharding: D axis (dim 2) split 4-ways per batch -> 8 slabs of 32 D-slices,
each with a 4-slice halo (host zero-pads volume edges).

Per core, on-chip bf16 with f32 PSUM accumulation:
  vols   : I, J, I*I, J*J, I*J                    (DVE / ACT)
  pass 1 : per-d-slice flip matmul vs 9-band B    -> H box,  [W, (43g, 128)]
  pass 2 : per-group flip matmul vs same B        -> W box,  [(l,d), (43g, 128)]
  pass 3 : weight-stationary block-band [128,96]  -> D box, f32 in PSUM
  pointwise cc + partial-sum accumulation         (DVE / ACT / GPSIMD)
Host: sum per-core partials, loss = -sum / N.

Group scheme: 43 groups of 3 h'-slices; groups 0..41 cover h' 0..125,
group 42 covers h' 125..127 (h'=125 duplicated, deduped in pass 3 by
using only loc 1:3 of the block band for the last chunk).
"""

import math

import numpy as np

import concourse.bacc as bacc
import concourse.bass as bass
import concourse.tile as tile
from concourse import mybir
from concourse.bass_utils import run_bass_kernel_spmd

F32 = mybir.dt.float32
BF16 = mybir.dt.bfloat16
ALU = mybir.AluOpType
ACTF = mybir.ActivationFunctionType

B, D, H, W = 2, 128, 128, 128
DL, PAD = 32, 4
DH = DL + 2 * PAD            # 40
NG = 43
C_SCALE = 32.0 / 729.0       # pass-3 band carries 1/32
EPS_P = 1e-5 / 1024.0
N_TOT = float(B * D * H * W)

_CACHE = {}


def _build():
    nc = bacc.Bacc(trn_type="TRN2", target_bir_lowering=False)

    i_dram = nc.dram_tensor("i_slab", [DH, H, W], F32, kind="ExternalInput")
    j_dram = nc.dram_tensor("j_slab", [DH, H, W], F32, kind="ExternalInput")
    out_dram = nc.dram_tensor("partials", [96, 1], F32, kind="ExternalOutput")

    with tile.TileContext(nc) as tc:
        with (
            tc.tile_pool(name="bands", bufs=1) as bands,
            tc.tile_pool(name="t2", bufs=1) as t2p,
            tc.tile_pool(name="accp", bufs=1) as accp,
            tc.tile_pool(name="ps12", bufs=3, space="PSUM") as ps12,
            tc.tile_pool(name="ps3", bufs=5, space="PSUM") as ps3p,
        ):
            # ---------- band matrices ----------
            # bh[p, j] = 1 iff |p - j| <= 4
            bh = bands.tile([128, 128], BF16)
            nc.gpsimd.memset(bh[:, :], 1.0)
            nc.gpsimd.affine_select(bh[:, :], bh[:, :], pattern=[[-1, 128]],
                                    compare_op=ALU.is_ge, fill=0.0,
                                    base=PAD, channel_multiplier=1)
            nc.gpsimd.affine_select(bh[:, :], bh[:, :], pattern=[[1, 128]],
                                    compare_op=ALU.is_ge, fill=0.0,
                                    base=PAD, channel_multiplier=-1)
            # b3[p, (l, j)] = 1/32 iff 0 <= p - 40l - j <= 8, rows 120+ zero
            b3 = bands.tile([128, 3, 32], BF16)
            nc.gpsimd.memset(b3[:, :, :], 1.0 / 32.0)
            nc.gpsimd.affine_select(b3[:, :, :], b3[:, :, :],
                                    pattern=[[-40, 3], [-1, 32]],
                                    compare_op=ALU.is_ge, fill=0.0,
                                    base=0, channel_multiplier=1)
            nc.gpsimd.affine_select(b3[:, :, :], b3[:, :, :],
                                    pattern=[[40, 3], [1, 32]],
                                    compare_op=ALU.is_ge, fill=0.0,
                                    base=8, channel_multiplier=-1)
            nc.gpsimd.affine_select(b3[:, :, :], b3[:, :, :],
                                    pattern=[[0, 3], [0, 32]],
                                    compare_op=ALU.is_ge, fill=0.0,
                                    base=119, channel_multiplier=-1)

            # ---------- inputs, chunked DMA [H, (D, W)] ----------
            inner = tc.tile_pool(name="inputs", bufs=1)
            inputs = inner.__enter__()
            innerv = tc.tile_pool(name="vols", bufs=2)
            volsp = innerv.__enter__()
            innert = tc.tile_pool(name="t1", bufs=2)
            t1p = innert.__enter__()
            i_f32 = inputs.tile([128, DH, W], F32)
            j_f32 = inputs.tile([128, DH, W], F32)
            i_re = i_dram.rearrange("d h w -> h d w")
            j_re = j_dram.rearrange("d h w -> h d w")
            for q in range(4):
                d0, d1 = q * 10, q * 10 + 10
                nc.sync.dma_start(out=i_f32[:, d0:d1, :], in_=i_re[:, d0:d1, :])
                nc.sync.dma_start(out=j_f32[:, d0:d1, :], in_=j_re[:, d0:d1, :])

            # ---------- t2 staging (all five live) ----------
            t2 = [t2p.tile([128, NG, 128], BF16, tag=f"t2_{v}", name=f"t2_{v}")
                  for v in range(5)]

            def make_vol(kind):
                v = volsp.tile([128, DH, W], BF16, tag="vol")
                for q in range(4):
                    s = slice(q * 10, q * 10 + 10)
                    if kind == "I":
                        nc.vector.tensor_copy(v[:, s, :], i_f32[:, s, :])
                    elif kind == "J":
                        nc.vector.tensor_copy(v[:, s, :], j_f32[:, s, :])
                    elif kind == "I2":
                        nc.scalar.square(v[:, s, :], i_f32[:, s, :])
                    elif kind == "J2":
                        nc.scalar.square(v[:, s, :], j_f32[:, s, :])
                    else:
                        nc.vector.tensor_tensor(out=v[:, s, :],
                                                in0=i_f32[:, s, :],
                                                in1=j_f32[:, s, :],
                                                op=ALU.mult)
                return v

            ncopy = 0
            for vi, kind in enumerate(["I", "J", "I2", "J2", "IJ"]):
                vol = make_vol(kind)

                # ----- pass 1: H filter ----- t1 = [W, (h', d)]
                t1 = t1p.tile([128, 128, DH], BF16, tag="t1")
                for db in range(10):
                    ps = ps12.tile([128, 4, 128], F32, tag="ps12")
                    for k in range(4):
                        nc.tensor.matmul(out=ps[:, k, :],
                                         lhsT=vol[:, db * 4 + k, :],
                                         rhs=bh[:, :])
                    dd = slice(db * 4, db * 4 + 4)
                    outA = t1[:, :, dd].rearrange("p h d -> p d h")
                    if ncopy % 2 == 0:
                        nc.scalar.copy(outA, ps[:, :, :])
                    else:
                        nc.vector.tensor_copy(outA, ps[:, :, :])
                    ncopy += 1

                # ----- pass 2: W filter -----
                # group g < 42: h' = 3g..3g+2 -> M = (l, d) = 120 rows
                # group 42: h' = 126, 127 -> M = 80 rows
                for gb in range(11):
                    gs = list(range(gb * 4, min(gb * 4 + 4, NG)))
                    ps = ps12.tile([128, 4, 128], F32, tag="ps12")
                    pmax = 0
                    for k, g in enumerate(gs):
                        h0, hn = (3 * g, 3) if g < 42 else (126, 2)
                        lhs = t1[:, h0:h0 + hn, :].rearrange(
                            "p l d -> p (l d)")
                        nc.tensor.matmul(out=ps[0:hn * DH, k, :],
                                         lhsT=lhs,
                                         rhs=bh[:, :])
                        pmax = max(pmax, hn * DH)
                    n = 2 if gb == 10 else len(gs)  # groups at 120 rows
                    if ncopy % 2 == 0:
                        nc.scalar.copy(t2[vi][0:120, gs[0]:gs[0] + n, :],
                                       ps[0:120, 0:n, :])
                    else:
                        nc.vector.tensor_copy(
                            t2[vi][0:120, gs[0]:gs[0] + n, :],
                            ps[0:120, 0:n, :])
                    ncopy += 1
                    if gb == 10:
                        nc.vector.tensor_copy(t2[vi][0:80, 42, :],
                                              ps[0:80, 2, :])

            innert.__exit__(None, None, None)
            innerv.__exit__(None, None, None)
            inner.__exit__(None, None, None)

            # ---------- pass 3 + chunked pointwise ----------
            cm_ptw = tc.tile_pool(name="ptw", bufs=2)
            ptw = cm_ptw.__enter__()
            acc_big = accp.tile([96, 512], F32)
            nc.vector.memset(acc_big[:, :], 0.0)
            accs = accp.tile([96, 1], F32)
            sqc = math.sqrt(C_SCALE)
            b3f = b3.rearrange("p l j -> p (l j)")

            for ci in range(12):
                if ci < 10:
                    g0, ng, P, F, Kk = ci * 4, 4, 96, 512, 120
                    lhs3 = b3f[0:120, 0:96]
                elif ci == 10:
                    g0, ng, P, F, Kk = 40, 2, 96, 256, 120
                    lhs3 = b3f[0:120, 0:96]
                else:
                    g0, ng, P, F, Kk = 42, 1, 64, 128, 80
                    lhs3 = b3f[0:80, 0:64]

                ps5 = []
                for v in range(5):
                    pt = ps3p.tile([96, 512], F32, tag="ps3")
                    nc.tensor.matmul(
                        out=pt[0:P, 0:F],
                        lhsT=lhs3,
                        rhs=t2[v][0:Kk, g0:g0 + ng, :].rearrange(
                            "p g w -> p (g w)"))
                    ps5.append(pt)
                psI = ps5[0][0:P, 0:F]
                psJ = ps5[1][0:P, 0:F]
                psI2 = ps5[2][0:P, 0:F]
                psJ2 = ps5[3][0:P, 0:F]
                psIJ = ps5[4][0:P, 0:F]

                qI = ptw.tile([96, 512], BF16, tag="qI", name="qI")[0:P, 0:F]
                qJ = ptw.tile([96, 512], BF16, tag="qJ", name="qJ")[0:P, 0:F]
                sJ = ptw.tile([96, 512], BF16, tag="sJ", name="sJ")[0:P, 0:F]
                nc.scalar.activation(qI, psI, ACTF.Square, scale=sqc)
                nc.scalar.activation(qJ, psJ, ACTF.Square, scale=sqc)
                nc.scalar.copy(sJ, psJ)

                m = ptw.tile([96, 512], BF16, tag="m", name="m")[0:P, 0:F]
                nc.vector.scalar_tensor_tensor(out=m, in0=psI, scalar=C_SCALE,
                                               in1=sJ, op0=ALU.mult,
                                               op1=ALU.mult)
                cross = ptw.tile([96, 512], BF16, tag="cross",
                                 name="cross")[0:P, 0:F]
                nc.vector.tensor_tensor(out=cross, in0=psIJ, in1=m,
                                        op=ALU.subtract)
                iv = ptw.tile([96, 512], BF16, tag="iv", name="iv")[0:P, 0:F]
                jv = ptw.tile([96, 512], BF16, tag="jv", name="jv")[0:P, 0:F]
                nc.vector.tensor_tensor(out=iv, in0=psI2, in1=qI,
                                        op=ALU.subtract)
                nc.vector.tensor_tensor(out=jv, in0=psJ2, in1=qJ,
                                        op=ALU.subtract)

                num = ptw.tile([96, 512], BF16, tag="num",
                               name="num")[0:P, 0:F]
                nc.scalar.activation(num, cross, ACTF.Square)

                den = ptw.tile([96, 512], BF16, tag="den",
                               name="den")[0:P, 0:F]
                nc.gpsimd.tensor_tensor(out=den, in0=iv, in1=jv, op=ALU.mult)
                dene = ptw.tile([96, 512], F32, tag="dene",
                                name="dene")[0:P, 0:F]
                nc.gpsimd.tensor_scalar(out=dene, in0=den, scalar1=EPS_P,
                                        scalar2=None, op0=ALU.add)
                rec = ptw.tile([96, 512], F32, tag="rec",
                               name="rec")[0:P, 0:F]
                nc.vector.reciprocal(out=rec, in_=dene)

                cc = ptw.tile([96, 512], BF16, tag="cc", name="cc")[0:P, 0:F]
                nc.gpsimd.tensor_tensor(out=cc, in0=num, in1=rec, op=ALU.mult)
                nc.gpsimd.tensor_tensor(out=acc_big[0:P, 0:F],
                                        in0=acc_big[0:P, 0:F], in1=cc,
                                        op=ALU.add)

            nc.vector.tensor_reduce(out=accs[:, :], in_=acc_big[:, :],
                                    axis=mybir.AxisListType.X, op=ALU.add)
            nc.sync.dma_start(out=out_dram[:, :], in_=accs[:, :])
            cm_ptw.__exit__(None, None, None)

    nc.compile()
    return nc


def kernel(y_pred: np.ndarray, y_true: np.ndarray) -> np.ndarray:
    y_pred = np.ascontiguousarray(np.asarray(y_pred, dtype=np.float32))
    y_true = np.ascontiguousarray(np.asarray(y_true, dtype=np.float32))

    if "nc" not in _CACHE:
        _CACHE["nc"] = _build()
    nc = _CACHE["nc"]

    in_maps = []
    for core in range(8):
        b = core // 4
        d0 = (core % 4) * DL
        islab = np.zeros((DH, H, W), np.float32)
        jslab = np.zeros((DH, H, W), np.float32)
        lo, hi = d0 - PAD, d0 + DL + PAD
        slo, shi = max(lo, 0), min(hi, D)
        islab[slo - lo:shi - lo] = y_true[b, 0, slo:shi]
        jslab[slo - lo:shi - lo] = y_pred[b, 0, slo:shi]
        in_maps.append({"i_slab": islab, "j_slab": jslab})

    res = run_bass_kernel_spmd(nc, in_maps, core_ids=list(range(8)))
    total = 0.0
    for r in res.results:
        total += float(np.asarray(r["partials"], np.float64).sum())
    return np.float32(-total / N_TOT)


if __name__ == "__main__":
    rng = np.random.default_rng(0)
    yp = rng.standard_normal((B, 1, D, H, W), dtype=np.float32)
    yt = rng.standard_normal((B, 1, D, H, W), dtype=np.float32)
    print("loss:", kernel(yp, yt))



# revision 8
# speedup vs baseline: 1.6458x; 1.6458x over previous
"""NCC loss (9x9x9 box normalized cross-correlation) on 8 TRN2 NeuronCores.

Inputs: y_pred, y_true f32 (2,1,128,128,128). Output: scalar f32 loss.

Sharding: D axis (dim 2) split 4-ways per batch -> 8 slabs of 32 D-slices,
each with a 4-slice halo (host zero-pads volume edges). Inputs are converted
to bf16 on the host (same precision as the on-device copy the previous
version did) and packed into a [104, 64, 128] layout: partitions 0..39 hold
the 40 halo'd d-rows for h-block 0 (h 0..63), partitions 64..103 hold them
for h-block 1 (h 64..127), rows 40..63 / 104..127 are zero.

Per core, separable box filter as three matmul passes (contract D, then W,
then H) so every intermediate is a full-128-partition tile:

  prep  : I*I, J*J, I*J products in bf16                  (DVE/ACT)
  P_D   : per h, lhsT=vol[d,w] slab, rhs=banded BD[40,32] -> t1 [w,(h,d')]
  P_W   : per d', lhsT=t1[w,h], rhs=band BW[128,128]      -> t2 [h,(d',w')]
  P_H   : stationary band BH, rhs=t2 chunks of 512        -> PSUM [h',512]
  ptw   : cc = cross^2/(I_var*J_var) with the three big PSUM subtractions
          done ON THE PE via accumulating -identity matmuls, reciprocal via
          the fast bit-trick custom DVE op, final mean via ones-matmul
          reduction accumulated in PSUM.
Host: sum per-core [128,8] partials, loss = -sum / N.
"""

import math

import numpy as np
import ml_dtypes

import concourse.bacc as bacc
import concourse.tile as tile
from concourse import mybir
from concourse.bass_utils import run_bass_kernel_spmd

F32 = mybir.dt.float32
BF16 = mybir.dt.bfloat16
ALU = mybir.AluOpType
ACTF = mybir.ActivationFunctionType

B, D, H, W = 2, 128, 128, 128
DL, PAD = 32, 4
DH = DL + 2 * PAD            # 40
SQS = math.sqrt(1.0 / 729.0)
N_TOT = float(B * D * H * W)

_CACHE = {}


def _build():
    nc = bacc.Bacc(trn_type="TRN2", target_bir_lowering=False)

    i_dram = nc.dram_tensor("i_pk", [104, 64, 128], BF16, kind="ExternalInput")
    j_dram = nc.dram_tensor("j_pk", [104, 64, 128], BF16, kind="ExternalInput")
    out_dram = nc.dram_tensor("partials", [128, 8], F32, kind="ExternalOutput")

    with tile.TileContext(nc) as tc:
        with (
            tc.tile_pool(name="bands", bufs=1) as bands,
            tc.tile_pool(name="stage", bufs=1) as stage,
            tc.tile_pool(name="accp", bufs=1) as accp,
        ):
            # ---------- band / constant matrices ----------
            # BD[p, j] = 1 iff j <= p <= j+8, duplicated at partition 64.
            bd = bands.tile([104, 32], BF16)
            nc.gpsimd.memset(bd[0:40, :], 1.0)
            nc.gpsimd.affine_select(bd[0:40, :], bd[0:40, :], pattern=[[-1, 32]],
                                    compare_op=ALU.is_ge, fill=0.0,
                                    base=0, channel_multiplier=1)
            nc.gpsimd.affine_select(bd[0:40, :], bd[0:40, :], pattern=[[1, 32]],
                                    compare_op=ALU.is_ge, fill=0.0,
                                    base=8, channel_multiplier=-1)
            nc.sync.dma_start(out=bd[64:104, :], in_=bd[0:40, :])

            # BW = BH: [p, j] = 1 iff |p - j| <= 4
            bw = bands.tile([128, 128], BF16)
            nc.gpsimd.memset(bw[:, :], 1.0)
            nc.gpsimd.affine_select(bw[:, :], bw[:, :], pattern=[[-1, 128]],
                                    compare_op=ALU.is_ge, fill=0.0,
                                    base=PAD, channel_multiplier=1)
            nc.gpsimd.affine_select(bw[:, :], bw[:, :], pattern=[[1, 128]],
                                    compare_op=ALU.is_ge, fill=0.0,
                                    base=PAD, channel_multiplier=-1)

            # -identity for PE-side subtraction
            negI = bands.tile([128, 128], BF16)
            nc.gpsimd.memset(negI[:, :], -1.0)
            nc.gpsimd.affine_select(negI[:, :], negI[:, :], pattern=[[-1, 128]],
                                    compare_op=ALU.is_ge, fill=0.0,
                                    base=0, channel_multiplier=1)
            nc.gpsimd.affine_select(negI[:, :], negI[:, :], pattern=[[1, 128]],
                                    compare_op=ALU.is_ge, fill=0.0,
                                    base=0, channel_multiplier=-1)

            ones = bands.tile([128, 1], BF16)
            nc.gpsimd.memset(ones[:, :], 1.0)

            # t2 tiles live until the end
            t2 = [stage.tile([128, 32, 128], BF16, name=f"t2_{v}")
                  for v in range(5)]

            # ---------- inputs + products ----------
            # pool stack (LIFO): t1 -> psD -> vols; vols popped after P_D.
            t1p = tc.tile_pool(name="t1", bufs=1)
            t1pool = t1p.__enter__()
            t1 = [t1pool.tile([128, 128, 32], BF16, name=f"t1_{v}")
                  for v in range(5)]
            psD = tc.tile_pool(name="psD", bufs=2, space="PSUM")
            psDp = psD.__enter__()
            volp = tc.tile_pool(name="vols", bufs=1)
            vols = volp.__enter__()
            vi = vols.tile([104, 64, 128], BF16, name="vi")
            vj = vols.tile([104, 64, 128], BF16, name="vj")
            vi2 = vols.tile([104, 64, 128], BF16, name="vi2")
            vj2 = vols.tile([104, 64, 128], BF16, name="vj2")
            vij = vols.tile([104, 64, 128], BF16, name="vij")
            for q in range(4):
                s = slice(q * 16, q * 16 + 16)
                nc.sync.dma_start(out=vi[:, s, :], in_=i_dram[:, s, :])
                nc.sync.dma_start(out=vj[:, s, :], in_=j_dram[:, s, :])
            for q in range(4):
                s = slice(q * 16, q * 16 + 16)
                nc.vector.tensor_tensor(out=vi2[:, s, :], in0=vi[:, s, :],
                                        in1=vi[:, s, :], op=ALU.mult)
                nc.scalar.square(vj2[:, s, :], vj[:, s, :])
                nc.vector.tensor_tensor(out=vij[:, s, :], in0=vi[:, s, :],
                                        in1=vj[:, s, :], op=ALU.mult)

            VOLS = [vi, vj, vi2, vj2, vij]

            # ---------- P_D: contract D -> t1 [w, (h 128, d' 32)] ----------
            # Pool cannot touch PSUM: evacuations alternate ACT / DVE only.
            nev = 0
            for v in range(5):
                vol = VOLS[v]
                for hb in range(4):           # 2-bank tiles of 32 h
                    ps = psDp.tile([128, 32, 32], F32, tag="psD")
                    for k in range(32):
                        h = hb * 32 + k
                        b, hl = h >> 6, h & 63
                        nc.tensor.matmul(
                            out=ps[:, k, :],
                            lhsT=vol[64 * b:64 * b + 40, hl, :],
                            rhs=bd[64 * b:64 * b + 40, :])
                    dst = t1[v][:, hb * 32:hb * 32 + 32, :]
                    if nev % 2 == 0:
                        nc.scalar.copy(dst, ps[:, :, :])
                    else:
                        nc.vector.tensor_copy(dst, ps[:, :, :])
                    nev += 1

            volp.__exit__(None, None, None)

            # ---------- P_W + P_H + pointwise, pipelined per d'-block ----
            # PSUM: psW bufs=3 (1 bank each) + psH 5 tiles... arranged so
            # P_W of block c+1 overlaps P_H+ptw of block c.
            psW = tc.tile_pool(name="psW", bufs=3, space="PSUM")
            psWp = psW.__enter__()
            for v in range(5):
                for db in range(8):           # banks of 4 d'
                    ps = psWp.tile([128, 4, 128], F32, tag="psW")
                    for k in range(4):
                        dp = db * 4 + k
                        nc.tensor.matmul(out=ps[:, k, :],
                                         lhsT=t1[v][:, :, dp],
                                         rhs=bw[:, :])
                    dst = t2[v][:, db * 4:db * 4 + 4, :]
                    if nev % 2 == 0:
                        nc.scalar.copy(dst, ps[:, :, :])
                    else:
                        nc.vector.tensor_copy(dst, ps[:, :, :])
                    nev += 1
            psW.__exit__(None, None, None)
            psD.__exit__(None, None, None)
            t1p.__exit__(None, None, None)

            # ---------- P_H + pointwise, 8 chunks of [128, 512] ----------
            psH = tc.tile_pool(name="psH", bufs=7, space="PSUM")
            psHp = psH.__enter__()
            psR = tc.tile_pool(name="psR", bufs=1, space="PSUM")
            psRp = psR.__enter__()
            ptwp = tc.tile_pool(name="ptw", bufs=2)
            ptw = ptwp.__enter__()

            acc_ps = psRp.tile([128, 8], F32)

            for c in range(8):
                rhs = [t2[v][:, c * 4:c * 4 + 4, :].rearrange("p a b -> p (a b)")
                       for v in range(5)]
                psI = psHp.tile([128, 512], F32, tag="psH")
                psJ = psHp.tile([128, 512], F32, tag="psH")
                nc.tensor.matmul(out=psI[:, :], lhsT=bw[:, :], rhs=rhs[0])
                nc.tensor.matmul(out=psJ[:, :], lhsT=bw[:, :], rhs=rhs[1])

                ap = ptw.tile([128, 512], BF16, tag="ap", name="ap")
                bp = ptw.tile([128, 512], BF16, tag="bp", name="bp")
                nc.scalar.mul(ap[:, :], psI[:, :], SQS)
                nc.scalar.mul(bp[:, :], psJ[:, :], SQS)

                qI = ptw.tile([128, 512], BF16, tag="qI", name="qI")
                qJ = ptw.tile([128, 512], BF16, tag="qJ", name="qJ")
                m = ptw.tile([128, 512], BF16, tag="m", name="m")
                nc.gpsimd.tensor_tensor(out=qI[:, :], in0=ap[:, :],
                                        in1=ap[:, :], op=ALU.mult)
                nc.gpsimd.tensor_tensor(out=qJ[:, :], in0=bp[:, :],
                                        in1=bp[:, :], op=ALU.mult)
                nc.vector.tensor_tensor(out=m[:, :], in0=ap[:, :],
                                        in1=bp[:, :], op=ALU.mult)

                psI2 = psHp.tile([128, 512], F32, tag="psH")
                psJ2 = psHp.tile([128, 512], F32, tag="psH")
                psIJ = psHp.tile([128, 512], F32, tag="psH")
                nc.tensor.matmul(out=psI2[:, :], lhsT=bw[:, :], rhs=rhs[2],
                                 start=True, stop=False)
                nc.tensor.matmul(out=psI2[:, :], lhsT=negI[:, :], rhs=qI[:, :],
                                 start=False, stop=True)
                nc.tensor.matmul(out=psJ2[:, :], lhsT=bw[:, :], rhs=rhs[3],
                                 start=True, stop=False)
                nc.tensor.matmul(out=psJ2[:, :], lhsT=negI[:, :], rhs=qJ[:, :],
                                 start=False, stop=True)
                nc.tensor.matmul(out=psIJ[:, :], lhsT=bw[:, :], rhs=rhs[4],
                                 start=True, stop=False)
                nc.tensor.matmul(out=psIJ[:, :], lhsT=negI[:, :], rhs=m[:, :],
                                 start=False, stop=True)

                ivp = ptw.tile([128, 512], BF16, tag="ivp", name="ivp")
                crp = ptw.tile([128, 512], BF16, tag="crp", name="crp")
                nc.scalar.copy(ivp[:, :], psI2[:, :])
                nc.scalar.copy(crp[:, :], psIJ[:, :])

                den = ptw.tile([128, 512], F32, tag="den", name="den")
                nc.vector.tensor_tensor(out=den[:, :], in0=psJ2[:, :],
                                        in1=ivp[:, :], op=ALU.mult)
                rec = ptw.tile([128, 512], F32, tag="rec", name="rec")
                nc.vector.reciprocal_approx_fast(out=rec[:, :], in_=den[:, :])

                t = ptw.tile([128, 512], BF16, tag="t", name="t")
                nc.vector.tensor_tensor(out=t[:, :], in0=crp[:, :],
                                        in1=rec[:, :], op=ALU.mult)
                cc = ptw.tile([128, 512], BF16, tag="cc", name="cc")
                nc.gpsimd.tensor_tensor(out=cc[:, :], in0=t[:, :],
                                        in1=crp[:, :], op=ALU.mult)

                for k in range(4):
                    nc.tensor.matmul(out=acc_ps[:, c:c + 1],
                                     lhsT=cc[:, 128 * k:128 * k + 128],
                                     rhs=ones[:, :],
                                     start=(k == 0), stop=(k == 3))

            accs = accp.tile([128, 8], F32)
            nc.scalar.copy(accs[:, :], acc_ps[:, :])
            nc.sync.dma_start(out=out_dram[:, :], in_=accs[:, :])
            ptwp.__exit__(None, None, None)
            psR.__exit__(None, None, None)
            psH.__exit__(None, None, None)

    nc.compile()
    return nc


def kernel(y_pred: np.ndarray, y_true: np.ndarray) -> np.ndarray:
    y_pred = np.asarray(y_pred, dtype=np.float32)
    y_true = np.asarray(y_true, dtype=np.float32)

    if "nc" not in _CACHE:
        _CACHE["nc"] = _build()
    nc = _CACHE["nc"]

    ib = y_true.astype(ml_dtypes.bfloat16)
    jb = y_pred.astype(ml_dtypes.bfloat16)

    in_maps = []
    for core in range(8):
        b = core // 4
        d0 = (core % 4) * DL
        lo, hi = d0 - PAD, d0 + DL + PAD
        slo, shi = max(lo, 0), min(hi, D)
        ipk = np.zeros((104, 64, 128), ml_dtypes.bfloat16)
        jpk = np.zeros((104, 64, 128), ml_dtypes.bfloat16)
        for hb in range(2):
            hs = slice(hb * 64, hb * 64 + 64)
            p0 = 64 * hb
            ipk[p0 + slo - lo:p0 + shi - lo] = ib[b, 0, slo:shi, hs, :]
            jpk[p0 + slo - lo:p0 + shi - lo] = jb[b, 0, slo:shi, hs, :]
        in_maps.append({"i_pk": ipk, "j_pk": jpk})

    res = run_bass_kernel_spmd(nc, in_maps, core_ids=list(range(8)))
    total = 0.0
    for r in res.results:
        total += float(np.asarray(r["partials"], np.float64).sum())
    return np.float32(-total / N_TOT)


if __name__ == "__main__":
    rng = np.random.default_rng(0)
    yp = rng.standard_normal((B, 1, D, H, W), dtype=np.float32)
    yt = rng.standard_normal((B, 1, D, H, W), dtype=np.float32)
    print("loss:", kernel(yp, yt))
